# revision 39
# baseline (speedup 1.0000x reference)
"""Trainium2 Bass kernel for nn_Center2D (DWT -> pool -> conv-BN-ReLU x2 -> deconv -> IDWT).

Self-contained: hardcodes shapes from the problem spec.
Sharding: pure data parallel, batch dim (8) across 8 cores; BN batch stats
synchronized with a tiny AllReduce (2x128 floats) per BN layer.

Layout strategy per core (one sample):
  front: PE matmul for DWT-H (contract H on partitions, banded matrix B1),
         DVE f16 even/odd taps for DWT-W, maxpool via DMA round-trip + TT-max,
  mid:   convs as 9 (resp. 6 K-packed) PE matmuls per output chunk, BN stats
         via accum_out during PSUM evacuation, BN+ReLU fused into one ACT op,
  back:  deconv as 4 PE matmuls, DRAM round-trip to put H on partitions,
         PE matmul for IDWT-H (banded matrix BH), DVE f16 taps for final
         IDWT-W writing interleaved, SWDGE cast f16->f32 on the output DMA.
"""

import os
import numpy as np

import concourse.bass as bass
import concourse.bacc as bacc
import concourse.tile as tile
from concourse import mybir
from concourse.bass_utils import run_bass_kernel_spmd

F32 = mybir.dt.float32
F16 = mybir.dt.float16
AF = mybir.ActivationFunctionType
ALU = mybir.AluOpType

REC = np.array([0.48296291314469025, 0.8365163037378079,
                0.22414386804185735, -0.12940952255092145], dtype=np.float64)
DEC = REC[::-1].copy()

N_CORES = int(os.environ.get("WK_CORES", "8"))
EPS = 1e-5


# ---------------------------------------------------------------- host consts
def build_B1():
    """DWT along H as a dense [256, 128] matrix (mirror edge folded in)."""
    B = np.zeros((256, 128), dtype=np.float64)
    for i in range(128):
        for idx, c in ((2*i-2, DEC[3]), (2*i-1, DEC[2]), (2*i, DEC[1]), (2*i+1, DEC[0])):
            if idx < 0:
                idx = -idx - 1
            B[idx, i] += c
    return B.astype(np.float32)


def build_BH():
    """IDWT along one axis as a dense [128, 254] matrix."""
    B = np.zeros((128, 254), dtype=np.float64)
    for t in range(127):
        B[t,   2*t] += REC[2]
        B[t+1, 2*t] += REC[0]
        B[t,   2*t+1] += REC[3]
        B[t+1, 2*t+1] += REC[1]
    return B.astype(np.float32)


def pack_consts(conv1_w, conv2_w, deconv_w, deconv_b, bn1_g, bn1_b, bn2_g, bn2_b):
    B1 = build_B1()
    b1p = np.zeros((128, 256), np.float32)
    b1p[:, 0:128] = B1[0:128, :]
    b1p[:, 128:256] = B1[128:256, :]

    bhw = build_BH().astype(np.float16)          # [128, 254]

    # conv1 packed K=128: rows (ky0|ky1, ci) for w1a; ky2 zero-padded to 128
    # rows in w1b so every tap runs at the fast K=128 rate.
    w1a = np.zeros((128, 3 * 128), np.float16)
    w1b = np.zeros((128, 3 * 128), np.float16)
    for kx in range(3):
        w1a[0:64,   kx*128:(kx+1)*128] = conv1_w[:, :, 0, kx].T
        w1a[64:128, kx*128:(kx+1)*128] = conv1_w[:, :, 1, kx].T
        w1b[0:64,   kx*128:(kx+1)*128] = conv1_w[:, :, 2, kx].T

    w2t = np.zeros((128, 9 * 128), np.float16)
    for ky in range(3):
        for kx in range(3):
            w2t[:, (ky*3+kx)*128:(ky*3+kx+1)*128] = conv2_w[:, :, ky, kx].T

    wdt = np.zeros((128, 4 * 64), np.float16)    # [ci, (k,l,o)]
    for k in range(2):
        for l in range(2):
            wdt[:, (k*2+l)*64:(k*2+l+1)*64] = deconv_w[:, :, k, l]

    # Transpose helper: E2[2t, t] = E2[2t+1, 64+t] = 1, so pw^T lands with
    # even DWT-H rows in PSUM cols 0:64 and odd rows in cols 64:128
    # (contiguous reads for the pool-H max instead of stride-2).
    e2 = np.zeros((128, 128), np.float16)
    for t in range(64):
        e2[2*t, t] = 1.0
        e2[2*t+1, 64+t] = 1.0

    return {
        "B1": b1p,
        "BHW": bhw,
        "EYE": e2,
        "w1a": w1a,
        "w1b": w1b,
        "w2t": w2t,
        "wdt": wdt,
        "db": deconv_b.reshape(64, 1).astype(np.float32),
        "bn1g": bn1_g.reshape(128, 1).astype(np.float32),
        "bn1b": bn1_b.reshape(128, 1).astype(np.float32),
        "bn2g": bn2_g.reshape(128, 1).astype(np.float32),
        "bn2b": bn2_b.reshape(128, 1).astype(np.float32),
    }


# ---------------------------------------------------------------- bass kernel
def build_nc(world=N_CORES, stage=None):
    if stage is None:
        stage = int(os.environ.get("WK_STAGE", "99"))
    nc = bacc.Bacc("TRN2", target_bir_lowering=False)
    use_cc = world > 1

    x = nc.dram_tensor("x", (64, 256, 256), F32, kind="ExternalInput")
    b1_d = nc.dram_tensor("B1", (128, 256), F32, kind="ExternalInput")
    bhw_d = nc.dram_tensor("BHW", (128, 254), F16, kind="ExternalInput")
    eye_d = nc.dram_tensor("EYE", (128, 128), F16, kind="ExternalInput")
    w1a_d = nc.dram_tensor("w1a", (128, 384), F16, kind="ExternalInput")
    w1b_d = nc.dram_tensor("w1b", (128, 384), F16, kind="ExternalInput")
    w2t_d = nc.dram_tensor("w2t", (128, 1152), F16, kind="ExternalInput")
    wdt_d = nc.dram_tensor("wdt", (128, 256), F16, kind="ExternalInput")
    db_d = nc.dram_tensor("db", (64, 1), F32, kind="ExternalInput")
    bn_vecs = {n: nc.dram_tensor(n, (128, 1), F32, kind="ExternalInput")
               for n in ("bn1g", "bn1b", "bn2g", "bn2b")}
    out_d = nc.dram_tensor("out", (254, 64, 254), F32, kind="ExternalOutput")

    scr2 = nc.dram_tensor("scr2", (64, 128, 128), F16, kind="Internal")
    cc_bufs = []
    for i in (1, 2):
        cc_bufs.append((
            nc.dram_tensor(f"bn{i}_in", (128, 2), F32, kind="Internal"),
            nc.dram_tensor(f"bn{i}_out", (128, 2), F32, kind="Internal",
                           addr_space="Shared"),
        ))
    rg = [list(range(world))]
    cnt = float(world * 64 * 64)

    with tile.TileContext(nc) as tc, \
         tc.tile_pool(name="persist", bufs=1) as pp:
        def _body():
            # ---------------- consts to SBUF
            b1_sb = pp.tile([128, 256], F32, name="b1_sb")
            nc.sync.dma_start(b1_sb[:], b1_d[:])
            bhw_sb = pp.tile([128, 254], F16, name="bhw_sb")
            nc.sync.dma_start(bhw_sb[:], bhw_d[:])
            eye_sb = pp.tile([128, 128], F16, name="eye_sb")
            nc.sync.dma_start(eye_sb[:], eye_d[:])
            w1a_sb = pp.tile([128, 384], F16, name="w1a_sb")
            nc.sync.dma_start(w1a_sb[:], w1a_d[:])
            w1b_sb = pp.tile([128, 384], F16, name="w1b_sb")
            nc.sync.dma_start(w1b_sb[:], w1b_d[:])
            w2t_sb = pp.tile([128, 1152], F16, name="w2t_sb")
            nc.sync.dma_start(w2t_sb[:], w2t_d[:])
            wdt_sb = pp.tile([128, 256], F16, name="wdt_sb")
            nc.sync.dma_start(wdt_sb[:], wdt_d[:])
            db_sb = pp.tile([64, 1], F32, name="db_sb")
            nc.sync.dma_start(db_sb[:], db_d[:])
            bnv = {}
            for n, d in bn_vecs.items():
                t = pp.tile([128, 1], F32, name=f"{n}_sb")
                nc.sync.dma_start(t[:], d[:])
                bnv[n] = t

            # conv1/conv2 inputs (live past their producing scopes); memset both
            # up front so the zeroing never sits on the critical path.
            # conv1 input, row-major, with partitions 64:128 holding the same
            # pooled image shifted up one row (ky-pair packing for K=128 taps).
            in1_pad = pp.tile([128, 66 * 66], F16, name="in1_pad")
            nc.vector.memset(in1_pad[:], 0.0)
            p1v = in1_pad[:].rearrange("p (r v) -> p r v", v=66)
            p1vT = in1_pad[:].rearrange("p (r v) -> p v r", v=66)
            in2_pad = pp.tile([128, 66 * 66], F16, name="in2_pad")
            nc.vector.memset(in2_pad[:], 0.0)
            p2v = in2_pad[:].rearrange("p (r v) -> p r v", v=66)

            # ---------------- front: DWT-H on PE, DWT-W on DVE, pool
            front_pool = tc.tile_pool(name="front", bufs=1)
            fp = front_pool.__enter__()
            xe_sb = fp.tile([128, 64 * 130], F16, name="xe_sb")
            xo_sb = fp.tile([128, 64 * 130], F16, name="xo_sb")
            xhb = x[:].rearrange("c h w -> h c w")   # [256, 64, 256]

            # mirror pad column 0:  xe[-1] = x[1] = xo[0],  xo[-1] = x[0] = xe[0]
            # DWT-W:  y[j] = D3 xe[j-1] + D2 xo[j-1] + D1 xe[j] + D0 xo[j]
            # DWT-W chunks (16 c each) are emitted INSIDE the load loop right
            # after the two chunks that feed them, so per-engine program order
            # matches dependency order (ACT runs its queue in order).
            xe_v = xe_sb[:].rearrange("p (c v) -> p c v", v=130)
            xo_v = xo_sb[:].rearrange("p (c v) -> p c v", v=130)
            y_sb = fp.tile([128, 64 * 128], F16, name="y_sb")
            tmp_sb = fp.tile([128, 64 * 128], F16, name="tmp_sb")
            y_v = y_sb[:].rearrange("p (c j) -> p c j", j=128)
            t_v = tmp_sb[:].rearrange("p (c j) -> p c j", j=128)
            pw_sb = fp.tile([128, 64 * 64], F16, name="pw_sb")
            pw_v = pw_sb[:].rearrange("p (c q) -> p c q", q=64)

            def dwtw_chunk(cc):
                sl = slice(cc * 16, (cc + 1) * 16)
                nc.vector.tensor_copy(xe_v[:, sl, 0:1], xo_v[:, sl, 1:2])
                nc.vector.tensor_copy(xo_v[:, sl, 0:1], xe_v[:, sl, 1:2])
                nc.scalar.activation(t_v[:, sl, :], xe_v[:, sl, 0:128], AF.Identity,
                                     scale=float(DEC[3]))
                nc.vector.scalar_tensor_tensor(t_v[:, sl, :], xo_v[:, sl, 0:128],
                                               float(DEC[2]), t_v[:, sl, :],
                                               ALU.mult, ALU.add)
                nc.vector.scalar_tensor_tensor(t_v[:, sl, :], xe_v[:, sl, 1:129],
                                               float(DEC[1]), t_v[:, sl, :],
                                               ALU.mult, ALU.add)
                nc.vector.scalar_tensor_tensor(y_v[:, sl, :], xo_v[:, sl, 1:129],
                                               float(DEC[0]), t_v[:, sl, :],
                                               ALU.mult, ALU.add)
                nc.vector.tensor_tensor(pw_v[:, sl, :], y_v[:, sl, 0::2],
                                        y_v[:, sl, 1::2], ALU.max)

            with tc.tile_pool(name="xin", bufs=3) as xin_pool, \
                 tc.tile_pool(name="psA", bufs=8, space="PSUM") as psA:
                for ci in range(8):          # c-chunks of 8 channels
                    c0 = ci * 8
                    xa = xin_pool.tile([128, 8 * 256], F32, tag="xa")
                    xb = xin_pool.tile([128, 8 * 256], F32, tag="xb")
                    nc.sync.dma_start(xa[:], xhb[0:128, c0:c0+8, :])
                    nc.scalar.dma_start(xb[:], xhb[128:256, c0:c0+8, :])
                    for half in range(2):     # 4 channels each
                        for par in range(2):  # even / odd w
                            ps = psA.tile([128, 512], F32, tag="psA")
                            rhs_a = xa[:].rearrange("p (c w) -> p c w", c=8)[
                                :, half*4:half*4+4, par::2]
                            rhs_b = xb[:].rearrange("p (c w) -> p c w", c=8)[
                                :, half*4:half*4+4, par::2]
                            nc.tensor.matmul(ps[:], b1_sb[:, 0:128], rhs_a,
                                             start=True, stop=False)
                            nc.tensor.matmul(ps[:], b1_sb[:, 128:256], rhs_b,
                                             start=False, stop=True)
                            dst = (xe_sb if par == 0 else xo_sb)[:].rearrange(
                                "p (c v) -> p c v", v=130)[:, c0+half*4:c0+half*4+4, 1:129]
                            if par == 0:
                                nc.vector.tensor_scalar(dst, ps[:], 1.0, None, ALU.mult)
                            else:
                                nc.scalar.copy(dst, ps[:])
                    if ci % 2 == 1:
                        dwtw_chunk(ci // 2)

            # PE-identity transpose pw [i, (q,c)] -> PSUM [c, i] per q, then
            # pool-H (max over i pairs) straight from PSUM into the padded
            # conv1 input. No DRAM round trip.
            with tc.tile_pool(name="psT", bufs=4, space="PSUM") as psT, \
                 tc.tile_pool(name="podd", bufs=4) as podd:
                for qg in range(16):
                    ps = psT.tile([64, 512], F32, tag="psT")
                    for qi in range(4):
                        q = qg * 4 + qi
                        nc.tensor.matmul(ps[:, qi*128:(qi+1)*128], pw_v[:, :, q],
                                         eye_sb[:], start=True, stop=True)
                    psv = ps[:].rearrange("p (q b t) -> p q b t", b=2, t=64)
                    # TT may read only one PSUM operand: stage odd rows via ACT
                    po = podd.tile([64, 256], F16, tag="po")
                    po_v = po[:].rearrange("p (q t) -> p q t", t=64)
                    nc.scalar.copy(po_v, psv[:, :, 1, :])
                    nc.vector.tensor_tensor(
                        p1vT[0:64, 1+qg*4:5+qg*4, 1:65],
                        psv[:, :, 0, :], po_v, ALU.max)
            # upper-partition duplicate, shifted up one row, via SBUF->SBUF DMA
            nc.sync.dma_start(in1_pad[64:128, 0:65*66], in1_pad[0:64, 66:66*66])
            front_pool.__exit__(None, None, None)
            if stage <= 2:
                return

            # ---------------- conv1 (+BN1 stats) ----------------
            mid_pool = tc.tile_pool(name="mid", bufs=1)
            mp = mid_pool.__enter__()
            a1_sb = mp.tile([128, 4096], F16, name="a1_sb")
            junk = pp.tile([128, 512], F32, name="junk")
            s1b = pp.tile([128, 8], F32, name="s1b")
            s2b = pp.tile([128, 8], F32, name="s2b")
            a1v = a1_sb[:].rearrange("p (r q) -> p r q", q=64)

            with tc.tile_pool(name="psB", bufs=8, space="PSUM") as psB:
                ps_list = [psB.tile([128, 512], F32, tag="psB", name=f"c1ps{i}")
                           for i in range(8)]
                # ch-halves: group 0's evacs/stats overlap group 1's matmuls
                for chg in range(2):
                    chs = range(chg * 4, chg * 4 + 4)
                    for ti in range(6):
                        grp, kx = divmod(ti, 3)
                        w_sb = w1a_sb if grp == 0 else w1b_sb
                        r_off = 0 if grp == 0 else 2
                        for ch in chs:
                            p0 = ch * 8
                            nc.tensor.matmul(ps_list[ch][:],
                                             w_sb[:, kx*128:(kx+1)*128],
                                             p1v[:, p0+r_off:p0+r_off+8, kx:kx+64],
                                             start=(ti == 0), stop=(ti == 5))
                    for ch in chs:
                        nc.vector.tensor_scalar(a1v[:, ch*8:ch*8+8, :], ps_list[ch][:],
                                                1.0, 0.0, ALU.mult, ALU.add,
                                                accum_out=s1b[:, ch:ch+1])
                        nc.scalar.activation(junk[:], ps_list[ch][:], AF.Square,
                                             accum_out=s2b[:, ch:ch+1])

            if stage <= 3:
                mid_pool.__exit__(None, None, None)
                return
            sc1, bi1 = _bn_coeffs(nc, pp, s1b, s2b, cc_bufs[0], rg, cnt,
                                  bnv["bn1g"], bnv["bn1b"], use_cc, tag=1)

            # BN1 + ReLU fused, written into padded conv2 input (2 chunks so
            # conv2's first taps can start while the second half applies)
            nc.scalar.activation(p2v[:, 1:33, 1:65], a1v[:, 0:32, :], AF.Relu,
                                 bias=bi1[:], scale=sc1[:])
            nc.scalar.activation(p2v[:, 33:65, 1:65], a1v[:, 32:64, :], AF.Relu,
                                 bias=bi1[:], scale=sc1[:])

            if stage <= 4:
                mid_pool.__exit__(None, None, None)
                return
            # ---------------- conv2 (+BN2 stats) ----------------
            h2_sb = mp.tile([128, 4096], F16, name="h2_sb")
            h2v = h2_sb[:].rearrange("p (r q) -> p r q", q=64)
            s1c = pp.tile([128, 8], F32, name="s1c")
            s2c = pp.tile([128, 8], F32, name="s2c")
            with tc.tile_pool(name="psC", bufs=8, space="PSUM") as psC:
                ps_list = [psC.tile([128, 512], F32, tag="psC", name=f"c2ps{i}")
                           for i in range(8)]
                for chg in range(2):
                    chs = range(chg * 4, chg * 4 + 4)
                    for ti in range(9):
                        ky, kx = divmod(ti, 3)
                        for ch in chs:
                            p0 = ch * 8
                            rhs = p2v[:, p0+ky:p0+ky+8, kx:kx+64]
                            nc.tensor.matmul(ps_list[ch][:], w2t_sb[:, ti*128:(ti+1)*128],
                                             rhs, start=(ti == 0), stop=(ti == 8))
                    for ch in chs:
                        nc.vector.tensor_scalar(h2v[:, ch*8:ch*8+8, :], ps_list[ch][:],
                                                1.0, 0.0, ALU.mult, ALU.add,
                                                accum_out=s1c[:, ch:ch+1])
                        nc.scalar.activation(junk[:], ps_list[ch][:], AF.Square,
                                             accum_out=s2c[:, ch:ch+1])

            sc2, bi2 = _bn_coeffs(nc, pp, s1c, s2c, cc_bufs[1], rg, cnt,
                                  bnv["bn2g"], bnv["bn2b"], use_cc, tag=2)
            nc.scalar.activation(h2v[:, 0:32, :], h2v[:, 0:32, :], AF.Relu,
                                 bias=bi2[:], scale=sc2[:])
            nc.scalar.activation(h2v[:, 32:64, :], h2v[:, 32:64, :], AF.Relu,
                                 bias=bi2[:], scale=sc2[:])

            if stage <= 5:
                mid_pool.__exit__(None, None, None)
                return
            # ---------------- deconv ----------------
            d_sb = mp.tile([64, 128 * 128], F16, name="d_sb")
            dv = d_sb[:].rearrange("p (h w) -> p h w", w=128)
            with tc.tile_pool(name="psD", bufs=8, space="PSUM") as psD:
                for kl in range(4):
                    k, l = divmod(kl, 2)
                    for ch in range(8):
                        p0 = ch * 8
                        ps = psD.tile([64, 512], F32, tag="psD")
                        nc.tensor.matmul(ps[:], wdt_sb[:, kl*64:(kl+1)*64],
                                         h2v[:, p0:p0+8, :], start=True, stop=True)
                        dst = dv[:, 2*p0+k:2*p0+k+15:2, l::2]
                        if (kl * 8 + ch) % 2 == 0:
                            nc.vector.tensor_scalar(dst, ps[:], 1.0, db_sb[:],
                                                    ALU.mult, ALU.add)
                        else:
                            nc.scalar.activation(dst, ps[:], AF.Identity,
                                                 bias=db_sb[:], scale=1.0)

            # round trip to put H on partitions
            for oq in range(4):
                eng = nc.sync if oq % 2 == 0 else nc.scalar
                eng.dma_start(scr2[:][oq*16:(oq+1)*16], dv[oq*16:(oq+1)*16])
            mid_pool.__exit__(None, None, None)
            if stage <= 6:
                return
            back_pool = tc.tile_pool(name="back", bufs=1)
            bp = back_pool.__enter__()
            dth = bp.tile([128, 64 * 128], F16, name="dth")
            dth_v = dth[:].rearrange("p (o w) -> p o w", w=128)
            scr2_h = scr2[:].rearrange("o h w -> h o w")
            for oq in range(4):
                eng = nc.sync if oq % 2 == 0 else nc.scalar
                eng.dma_start(dth_v[:, oq*16:(oq+1)*16, :],
                              scr2_h[:, oq*16:(oq+1)*16, :])

            # ---------------- IDWT-H on PE, IDWT-W on DVE ----------------
            # Two overlapping 128-row blocks (rows 0:128 and 126:254): HWDGE only
            # stripes a DMA across the 16 SDMA engines when the SBUF partition
            # count is a multiple of 16; 127-row stores serialize on one engine.
            with tc.tile_pool(name="psE", bufs=8, space="PSUM") as psE, \
                 tc.tile_pool(name="gpool", bufs=2) as gpool, \
                 tc.tile_pool(name="twpool", bufs=1) as twpool, \
                 tc.tile_pool(name="opool", bufs=1) as opool:
                for blk in range(2):
                    r0 = 0 if blk == 0 else 126
                    g_t = gpool.tile([128, 8192], F16, tag="g")
                    g_v = g_t[:].rearrange("p (o w) -> p o w", w=128)
                    for nch in range(16):
                        ps = psE.tile([128, 512], F32, tag="psE")
                        nc.tensor.matmul(ps[:], bhw_sb[:, r0:r0+128],
                                         dth[:, nch*512:(nch+1)*512],
                                         start=True, stop=True)
                        dst = g_t[:, nch*512:(nch+1)*512]
                        if nch % 2 == 0:
                            nc.vector.tensor_copy(dst, ps[:])
                        else:
                            nc.scalar.copy(dst, ps[:])
                    # IDWT-W chunked by o (4 x 16 channels): ACT produces the
                    # scaled copies, DVE interleaves, and each chunk's store
                    # starts while the next chunk computes.
                    o_t = opool.tile([128, 64 * 254], F32, tag="o")
                    o_v = o_t[:].rearrange("p (o w) -> p o w", w=254)
                    tw = twpool.tile([128, 8192], F16, tag="tw")
                    tw_v = tw[:].rearrange("p (o w) -> p o w", w=128)
                    twb = twpool.tile([128, 8192], F16, tag="twb")
                    twb_v = twb[:].rearrange("p (o w) -> p o w", w=128)
                    for oc in range(4):
                        osl = slice(oc * 16, (oc + 1) * 16)
                        nc.scalar.activation(tw_v[:, osl, :], g_v[:, osl, :],
                                             AF.Identity, scale=float(REC[2]))
                        nc.scalar.activation(twb_v[:, osl, :], g_v[:, osl, :],
                                             AF.Identity, scale=float(REC[3]))
                        nc.vector.scalar_tensor_tensor(
                            o_v[:, osl, 0:253:2], g_v[:, osl, 1:128], float(REC[0]),
                            tw_v[:, osl, 0:127], ALU.mult, ALU.add)
                        nc.vector.scalar_tensor_tensor(
                            o_v[:, osl, 1:254:2], g_v[:, osl, 1:128], float(REC[1]),
                            twb_v[:, osl, 0:127], ALU.mult, ALU.add)
                        eng = nc.sync if oc % 2 == 0 else nc.scalar
                        eng.dma_start(out_d[:][r0:r0+128, osl, :], o_v[:, osl, :])
            back_pool.__exit__(None, None, None)

        _body()
    nc.compile()
    return nc


def _bn_coeffs(nc, pp, s1b, s2b, cc_pair, rg, cnt, g_sb, b_sb, use_cc, tag):
    """Reduce per-chunk sums, AllReduce across cores, return (scale, bias) [128,1]."""
    ALU = mybir.AluOpType
    sl = pp.tile([128, 2], F32, name=f"bn{tag}_sl")
    nc.vector.tensor_reduce(sl[:, 0:1], s1b[:], mybir.AxisListType.X, ALU.add)
    nc.vector.tensor_reduce(sl[:, 1:2], s2b[:], mybir.AxisListType.X, ALU.add)
    cc_in, cc_out = cc_pair
    sg = pp.tile([128, 2], F32, name=f"bn{tag}_sg")
    if use_cc:
        nc.sync.dma_start(cc_in[:], sl[:])
        nc.gpsimd.collective_compute(
            "AllReduce", ALU.add, replica_groups=rg,
            ins=[cc_in[:]], outs=[cc_out[:]])
        nc.sync.dma_start(sg[:], cc_out[:])
    else:
        nc.vector.tensor_copy(sg[:], sl[:])

    m = pp.tile([128, 1], F32, name=f"bn{tag}_m")
    vpe = pp.tile([128, 1], F32, name=f"bn{tag}_v")
    t0 = pp.tile([128, 1], F32, name=f"bn{tag}_t0")
    y0 = pp.tile([128, 1], F32, name=f"bn{tag}_y0")
    nc.vector.tensor_scalar(m[:], sg[:, 0:1], 1.0 / cnt, None, ALU.mult)
    nc.vector.tensor_tensor(t0[:], m[:], m[:], ALU.mult)          # m^2
    nc.vector.tensor_scalar(vpe[:], sg[:, 1:2], 1.0 / cnt, float(EPS), ALU.mult,
                            ALU.add)                              # E[x^2]+eps
    nc.vector.tensor_tensor(vpe[:], vpe[:], t0[:], ALU.subtract)  # var+eps
    s0 = pp.tile([128, 1], F32, name=f"bn{tag}_s0")
    nc.scalar.activation(s0[:], vpe[:], mybir.ActivationFunctionType.Sqrt)
    nc.vector.reciprocal(y0[:], s0[:])
    sc = pp.tile([128, 1], F32, name=f"bn{tag}_sc")
    bi = pp.tile([128, 1], F32, name=f"bn{tag}_bi")
    nc.vector.tensor_tensor(sc[:], y0[:], g_sb[:], ALU.mult)
    nc.vector.tensor_tensor(t0[:], m[:], sc[:], ALU.mult)
    nc.vector.tensor_tensor(bi[:], b_sb[:], t0[:], ALU.subtract)
    return sc, bi


# ---------------------------------------------------------------- entry point
_CACHE = {}


def kernel(x, conv1_w, conv1_b, bn1_g, bn1_b, conv2_w, conv2_b, bn2_g, bn2_b,
           deconv_w, deconv_b):
    world = N_CORES
    if "nc" not in _CACHE:
        _CACHE["nc"] = build_nc(world)
    nc = _CACHE["nc"]

    consts = pack_consts(np.asarray(conv1_w), np.asarray(conv2_w),
                         np.asarray(deconv_w), np.asarray(deconv_b),
                         np.asarray(bn1_g), np.asarray(bn1_b),
                         np.asarray(bn2_g), np.asarray(bn2_b))
    x = np.asarray(x)
    in_maps = []
    for n in range(world):
        m = {"x": np.ascontiguousarray(x[n])}
        m.update(consts)
        in_maps.append(m)

    res = run_bass_kernel_spmd(
        nc, in_maps, core_ids=list(range(world)),
        trace=bool(int(os.environ.get("WK_TRACE", "0"))))
    out = np.stack([r["out"].transpose(1, 0, 2) for r in res.results], axis=0)
    _CACHE["last_perf"] = res
    return out



# revision 43
# speedup vs baseline: 1.0276x; 1.0276x over previous
"""Trainium2 Bass kernel for nn_Center2D (DWT -> pool -> conv-BN-ReLU x2 -> deconv -> IDWT).

Self-contained: hardcodes shapes from the problem spec.
Sharding: pure data parallel, batch dim (8) across 8 cores; BN batch stats
synchronized with a tiny AllReduce (2x128 floats) per BN layer.

Layout strategy per core (one sample):
  front: PE matmul for DWT-H (contract H on partitions, banded matrix B1),
         DVE f16 taps for DWT-W chunked by c and interleaved with the x DMAs,
         pool-W on DVE, then PE-identity transposes (permuted eye: even/odd
         rows land in separate PSUM blocks) + pool-H max from PSUM; the
         row-shifted upper-partition copy for K=128 conv1 taps is one
         SBUF->SBUF DMA.  No DRAM round trip.
  mid:   convs as 6/9 K=128-packed PE matmuls per output chunk (K=64 matmuls
         are ~2.2x slower per column - avoid), BN stats via accum_out during
         PSUM evacuation, tiny AllReduce per BN, BN+ReLU fused ACT ops (2
         chunks each so the next conv starts early).
  back:  deconv as 4 PE matmuls, DRAM round-trip for H-on-partitions,
         PE matmul for IDWT-H (two overlapping 128-row blocks -> every
         output store has a 16-divisible partition count and stripes
         across all 16 SDMA engines; 127-partition stores serialize on
         ONE engine at 27 GB/s), IDWT-W as ACT-scaled copies + DVE
         interleaved adds, chunked by o with the store of each chunk
         overlapping the next chunk's compute.
"""

import os
import numpy as np

import concourse.bass as bass
import concourse.bacc as bacc
import concourse.tile as tile
from concourse import mybir
from concourse.bass_utils import run_bass_kernel_spmd

F32 = mybir.dt.float32
F16 = mybir.dt.float16
AF = mybir.ActivationFunctionType
ALU = mybir.AluOpType

REC = np.array([0.48296291314469025, 0.8365163037378079,
                0.22414386804185735, -0.12940952255092145], dtype=np.float64)
DEC = REC[::-1].copy()

N_CORES = int(os.environ.get("WK_CORES", "8"))
EPS = 1e-5


# ---------------------------------------------------------------- host consts
def build_B1():
    """DWT along H as a dense [256, 128] matrix (mirror edge folded in)."""
    B = np.zeros((256, 128), dtype=np.float64)
    for i in range(128):
        for idx, c in ((2*i-2, DEC[3]), (2*i-1, DEC[2]), (2*i, DEC[1]), (2*i+1, DEC[0])):
            if idx < 0:
                idx = -idx - 1
            B[idx, i] += c
    return B.astype(np.float32)


def build_BH():
    """IDWT along one axis as a dense [128, 254] matrix."""
    B = np.zeros((128, 254), dtype=np.float64)
    for t in range(127):
        B[t,   2*t] += REC[2]
        B[t+1, 2*t] += REC[0]
        B[t,   2*t+1] += REC[3]
        B[t+1, 2*t+1] += REC[1]
    return B.astype(np.float32)


def pack_consts(conv1_w, conv2_w, deconv_w, deconv_b, bn1_g, bn1_b, bn2_g, bn2_b):
    B1 = build_B1()
    b1p = np.zeros((128, 256), np.float32)
    b1p[:, 0:128] = B1[0:128, :]
    b1p[:, 128:256] = B1[128:256, :]

    bhw = build_BH().astype(np.float16)          # [128, 254]

    # conv1 packed K=128: rows (ky0|ky1, ci) for w1a; ky2 zero-padded to 128
    # rows in w1b so every tap runs at the fast K=128 rate.
    w1a = np.zeros((128, 3 * 128), np.float16)
    w1b = np.zeros((128, 3 * 128), np.float16)
    for kx in range(3):
        w1a[0:64,   kx*128:(kx+1)*128] = conv1_w[:, :, 0, kx].T
        w1a[64:128, kx*128:(kx+1)*128] = conv1_w[:, :, 1, kx].T
        w1b[0:64,   kx*128:(kx+1)*128] = conv1_w[:, :, 2, kx].T

    w2t = np.zeros((128, 9 * 128), np.float16)
    for ky in range(3):
        for kx in range(3):
            w2t[:, (ky*3+kx)*128:(ky*3+kx+1)*128] = conv2_w[:, :, ky, kx].T

    wdt = np.zeros((128, 4 * 64), np.float16)    # [ci, (k,l,o)]
    for k in range(2):
        for l in range(2):
            wdt[:, (k*2+l)*64:(k*2+l+1)*64] = deconv_w[:, :, k, l]

    # Transpose helper: E2[2t, t] = E2[2t+1, 64+t] = 1, so pw^T lands with
    # even DWT-H rows in PSUM cols 0:64 and odd rows in cols 64:128
    # (contiguous reads for the pool-H max instead of stride-2).
    e2 = np.zeros((128, 128), np.float16)
    for t in range(64):
        e2[2*t, t] = 1.0
        e2[2*t+1, 64+t] = 1.0

    return {
        "B1": b1p,
        "BHW": bhw,
        "EYE": e2,
        "w1a": w1a,
        "w1b": w1b,
        "w2t": w2t,
        "wdt": wdt,
        "db": deconv_b.reshape(64, 1).astype(np.float32),
        "bn1g": bn1_g.reshape(128, 1).astype(np.float32),
        "bn1b": bn1_b.reshape(128, 1).astype(np.float32),
        "bn2g": bn2_g.reshape(128, 1).astype(np.float32),
        "bn2b": bn2_b.reshape(128, 1).astype(np.float32),
    }


# ---------------------------------------------------------------- bass kernel
def build_nc(world=N_CORES, stage=None):
    if stage is None:
        stage = int(os.environ.get("WK_STAGE", "99"))
    nc = bacc.Bacc("TRN2", target_bir_lowering=False)
    use_cc = world > 1

    x = nc.dram_tensor("x", (64, 256, 256), F32, kind="ExternalInput")
    b1_d = nc.dram_tensor("B1", (128, 256), F32, kind="ExternalInput")
    bhw_d = nc.dram_tensor("BHW", (128, 254), F16, kind="ExternalInput")
    eye_d = nc.dram_tensor("EYE", (128, 128), F16, kind="ExternalInput")
    w1a_d = nc.dram_tensor("w1a", (128, 384), F16, kind="ExternalInput")
    w1b_d = nc.dram_tensor("w1b", (128, 384), F16, kind="ExternalInput")
    w2t_d = nc.dram_tensor("w2t", (128, 1152), F16, kind="ExternalInput")
    wdt_d = nc.dram_tensor("wdt", (128, 256), F16, kind="ExternalInput")
    db_d = nc.dram_tensor("db", (64, 1), F32, kind="ExternalInput")
    bn_vecs = {n: nc.dram_tensor(n, (128, 1), F32, kind="ExternalInput")
               for n in ("bn1g", "bn1b", "bn2g", "bn2b")}
    out_d = nc.dram_tensor("out", (254, 64, 254), F32, kind="ExternalOutput")

    scr2 = nc.dram_tensor("scr2", (64, 128, 128), F16, kind="Internal")
    cc_bufs = []
    for i in (1, 2):
        cc_bufs.append((
            nc.dram_tensor(f"bn{i}_in", (128, 2), F32, kind="Internal"),
            nc.dram_tensor(f"bn{i}_out", (128, 2), F32, kind="Internal",
                           addr_space="Shared"),
        ))
    rg = [list(range(world))]
    cnt = float(world * 64 * 64)

    with tile.TileContext(nc) as tc, \
         tc.tile_pool(name="persist", bufs=1) as pp:
        def _body():
            # ---------------- consts to SBUF
            b1_sb = pp.tile([128, 256], F32, name="b1_sb")
            nc.sync.dma_start(b1_sb[:], b1_d[:])
            bhw_sb = pp.tile([128, 254], F16, name="bhw_sb")
            nc.sync.dma_start(bhw_sb[:], bhw_d[:])
            eye_sb = pp.tile([128, 128], F16, name="eye_sb")
            nc.sync.dma_start(eye_sb[:], eye_d[:])
            w1a_sb = pp.tile([128, 384], F16, name="w1a_sb")
            nc.sync.dma_start(w1a_sb[:], w1a_d[:])
            w1b_sb = pp.tile([128, 384], F16, name="w1b_sb")
            nc.sync.dma_start(w1b_sb[:], w1b_d[:])
            w2t_sb = pp.tile([128, 1152], F16, name="w2t_sb")
            nc.sync.dma_start(w2t_sb[:], w2t_d[:])
            wdt_sb = pp.tile([128, 256], F16, name="wdt_sb")
            nc.sync.dma_start(wdt_sb[:], wdt_d[:])
            db_sb = pp.tile([64, 1], F32, name="db_sb")
            nc.sync.dma_start(db_sb[:], db_d[:])
            bnv = {}
            for n, d in bn_vecs.items():
                t = pp.tile([128, 1], F32, name=f"{n}_sb")
                nc.sync.dma_start(t[:], d[:])
                bnv[n] = t

            # conv1/conv2 inputs (live past their producing scopes); memset both
            # up front so the zeroing never sits on the critical path.
            # conv1 input, row-major, with partitions 64:128 holding the same
            # pooled image shifted up one row (ky-pair packing for K=128 taps).
            in1_pad = pp.tile([128, 66 * 66], F16, name="in1_pad")
            nc.vector.memset(in1_pad[:], 0.0)
            p1v = in1_pad[:].rearrange("p (r v) -> p r v", v=66)
            p1vT = in1_pad[:].rearrange("p (r v) -> p v r", v=66)
            in2_pad = pp.tile([128, 66 * 66], F16, name="in2_pad")
            nc.vector.memset(in2_pad[:], 0.0)
            p2v = in2_pad[:].rearrange("p (r v) -> p r v", v=66)

            # ---------------- front: DWT-H on PE, DWT-W on DVE, pool
            front_pool = tc.tile_pool(name="front", bufs=1)
            fp = front_pool.__enter__()
            xe_sb = fp.tile([128, 64 * 130], F16, name="xe_sb")
            xo_sb = fp.tile([128, 64 * 130], F16, name="xo_sb")
            xhb = x[:].rearrange("c h w -> h c w")   # [256, 64, 256]

            # mirror pad column 0:  xe[-1] = x[1] = xo[0],  xo[-1] = x[0] = xe[0]
            # DWT-W:  y[j] = D3 xe[j-1] + D2 xo[j-1] + D1 xe[j] + D0 xo[j]
            # DWT-W chunks (16 c each) are emitted INSIDE the load loop right
            # after the two chunks that feed them, so per-engine program order
            # matches dependency order (ACT runs its queue in order).
            xe_v = xe_sb[:].rearrange("p (c v) -> p c v", v=130)
            xo_v = xo_sb[:].rearrange("p (c v) -> p c v", v=130)
            y_sb = fp.tile([128, 64 * 128], F16, name="y_sb")
            tmp_sb = fp.tile([128, 64 * 128], F16, name="tmp_sb")
            y_v = y_sb[:].rearrange("p (c j) -> p c j", j=128)
            t_v = tmp_sb[:].rearrange("p (c j) -> p c j", j=128)
            pw_sb = fp.tile([128, 64 * 64], F16, name="pw_sb")
            pw_v = pw_sb[:].rearrange("p (c q) -> p c q", q=64)

            def dwtw_chunk(cc):
                sl = slice(cc * 16, (cc + 1) * 16)
                nc.vector.tensor_copy(xe_v[:, sl, 0:1], xo_v[:, sl, 1:2])
                nc.vector.tensor_copy(xo_v[:, sl, 0:1], xe_v[:, sl, 1:2])
                nc.scalar.activation(t_v[:, sl, :], xe_v[:, sl, 0:128], AF.Identity,
                                     scale=float(DEC[3]))
                nc.vector.scalar_tensor_tensor(t_v[:, sl, :], xo_v[:, sl, 0:128],
                                               float(DEC[2]), t_v[:, sl, :],
                                               ALU.mult, ALU.add)
                nc.vector.scalar_tensor_tensor(t_v[:, sl, :], xe_v[:, sl, 1:129],
                                               float(DEC[1]), t_v[:, sl, :],
                                               ALU.mult, ALU.add)
                nc.vector.scalar_tensor_tensor(y_v[:, sl, :], xo_v[:, sl, 1:129],
                                               float(DEC[0]), t_v[:, sl, :],
                                               ALU.mult, ALU.add)
                nc.vector.tensor_tensor(pw_v[:, sl, :], y_v[:, sl, 0::2],
                                        y_v[:, sl, 1::2], ALU.max)

            with tc.tile_pool(name="xin", bufs=3) as xin_pool, \
                 tc.tile_pool(name="psA", bufs=8, space="PSUM") as psA:
                for ci in range(8):          # c-chunks of 8 channels
                    c0 = ci * 8
                    xa = xin_pool.tile([128, 8 * 256], F32, tag="xa")
                    xb = xin_pool.tile([128, 8 * 256], F32, tag="xb")
                    nc.sync.dma_start(xa[:], xhb[0:128, c0:c0+8, :])
                    nc.scalar.dma_start(xb[:], xhb[128:256, c0:c0+8, :])
                    for half in range(2):     # 4 channels each
                        for par in range(2):  # even / odd w
                            ps = psA.tile([128, 512], F32, tag="psA")
                            rhs_a = xa[:].rearrange("p (c w) -> p c w", c=8)[
                                :, half*4:half*4+4, par::2]
                            rhs_b = xb[:].rearrange("p (c w) -> p c w", c=8)[
                                :, half*4:half*4+4, par::2]
                            nc.tensor.matmul(ps[:], b1_sb[:, 0:128], rhs_a,
                                             start=True, stop=False)
                            nc.tensor.matmul(ps[:], b1_sb[:, 128:256], rhs_b,
                                             start=False, stop=True)
                            dst = (xe_sb if par == 0 else xo_sb)[:].rearrange(
                                "p (c v) -> p c v", v=130)[:, c0+half*4:c0+half*4+4, 1:129]
                            if par == 0:
                                nc.vector.tensor_scalar(dst, ps[:], 1.0, None, ALU.mult)
                            else:
                                nc.scalar.copy(dst, ps[:])
                    if ci % 2 == 1:
                        dwtw_chunk(ci // 2)

            # PE-identity transpose pw [i, (q,c)] -> PSUM [c, i] per q, then
            # pool-H (max over i pairs) straight from PSUM into the padded
            # conv1 input. No DRAM round trip.
            with tc.tile_pool(name="psT", bufs=4, space="PSUM") as psT, \
                 tc.tile_pool(name="podd", bufs=4) as podd:
                for qg in range(16):
                    ps = psT.tile([64, 512], F32, tag="psT")
                    for qi in range(4):
                        q = qg * 4 + qi
                        nc.tensor.matmul(ps[:, qi*128:(qi+1)*128], pw_v[:, :, q],
                                         eye_sb[:], start=True, stop=True)
                    psv = ps[:].rearrange("p (q b t) -> p q b t", b=2, t=64)
                    # TT may read only one PSUM operand: stage odd rows via ACT
                    po = podd.tile([64, 256], F16, tag="po")
                    po_v = po[:].rearrange("p (q t) -> p q t", t=64)
                    nc.scalar.copy(po_v, psv[:, :, 1, :])
                    nc.vector.tensor_tensor(
                        p1vT[0:64, 1+qg*4:5+qg*4, 1:65],
                        psv[:, :, 0, :], po_v, ALU.max)
            # upper-partition duplicate, shifted up one row, via SBUF->SBUF DMA
            nc.sync.dma_start(in1_pad[64:128, 0:65*66], in1_pad[0:64, 66:66*66])
            front_pool.__exit__(None, None, None)
            if stage <= 2:
                return

            # ---------------- conv1 (+BN1 stats) ----------------
            mid_pool = tc.tile_pool(name="mid", bufs=1)
            mp = mid_pool.__enter__()
            a1_sb = mp.tile([128, 4096], F16, name="a1_sb")
            junk = pp.tile([128, 512], F32, name="junk")
            s1b = pp.tile([128, 8], F32, name="s1b")
            s2b = pp.tile([128, 8], F32, name="s2b")
            a1v = a1_sb[:].rearrange("p (r q) -> p r q", q=64)

            with tc.tile_pool(name="psB", bufs=8, space="PSUM") as psB:
                ps_list = [psB.tile([128, 512], F32, tag="psB", name=f"c1ps{i}")
                           for i in range(8)]
                for ti in range(6):
                    grp, kx = divmod(ti, 3)
                    w_sb = w1a_sb if grp == 0 else w1b_sb
                    r_off = 0 if grp == 0 else 2
                    for ch in range(8):
                        p0 = ch * 8
                        nc.tensor.matmul(ps_list[ch][:],
                                         w_sb[:, kx*128:(kx+1)*128],
                                         p1v[:, p0+r_off:p0+r_off+8, kx:kx+64],
                                         start=(ti == 0), stop=(ti == 5))
                for ch in range(8):
                    nc.vector.tensor_scalar(a1v[:, ch*8:ch*8+8, :], ps_list[ch][:],
                                            1.0, 0.0, ALU.mult, ALU.add,
                                            accum_out=s1b[:, ch:ch+1])
                    nc.scalar.activation(junk[:], ps_list[ch][:], AF.Square,
                                         accum_out=s2b[:, ch:ch+1])

            if stage <= 3:
                mid_pool.__exit__(None, None, None)
                return
            sc1, bi1 = _bn_coeffs(nc, pp, s1b, s2b, cc_bufs[0], rg, cnt,
                                  bnv["bn1g"], bnv["bn1b"], use_cc, tag=1)

            # BN1 + ReLU fused, written into padded conv2 input (2 chunks so
            # conv2's first taps can start while the second half applies)
            nc.scalar.activation(p2v[:, 1:33, 1:65], a1v[:, 0:32, :], AF.Relu,
                                 bias=bi1[:], scale=sc1[:])
            nc.scalar.activation(p2v[:, 33:65, 1:65], a1v[:, 32:64, :], AF.Relu,
                                 bias=bi1[:], scale=sc1[:])

            if stage <= 4:
                mid_pool.__exit__(None, None, None)
                return
            # ---------------- conv2 (+BN2 stats) ----------------
            h2_sb = mp.tile([128, 4096], F16, name="h2_sb")
            h2v = h2_sb[:].rearrange("p (r q) -> p r q", q=64)
            s1c = pp.tile([128, 8], F32, name="s1c")
            s2c = pp.tile([128, 8], F32, name="s2c")
            with tc.tile_pool(name="psC", bufs=8, space="PSUM") as psC:
                ps_list = [psC.tile([128, 512], F32, tag="psC", name=f"c2ps{i}")
                           for i in range(8)]
                for ti in range(9):
                    ky, kx = divmod(ti, 3)
                    for ch in range(8):
                        p0 = ch * 8
                        rhs = p2v[:, p0+ky:p0+ky+8, kx:kx+64]
                        nc.tensor.matmul(ps_list[ch][:], w2t_sb[:, ti*128:(ti+1)*128],
                                         rhs, start=(ti == 0), stop=(ti == 8))
                for ch in range(8):
                    nc.vector.tensor_scalar(h2v[:, ch*8:ch*8+8, :], ps_list[ch][:],
                                            1.0, 0.0, ALU.mult, ALU.add,
                                            accum_out=s1c[:, ch:ch+1])
                    nc.scalar.activation(junk[:], ps_list[ch][:], AF.Square,
                                         accum_out=s2c[:, ch:ch+1])

            sc2, bi2 = _bn_coeffs(nc, pp, s1c, s2c, cc_bufs[1], rg, cnt,
                                  bnv["bn2g"], bnv["bn2b"], use_cc, tag=2)
            nc.scalar.activation(h2v[:, 0:32, :], h2v[:, 0:32, :], AF.Relu,
                                 bias=bi2[:], scale=sc2[:])
            nc.scalar.activation(h2v[:, 32:64, :], h2v[:, 32:64, :], AF.Relu,
                                 bias=bi2[:], scale=sc2[:])

            if stage <= 5:
                mid_pool.__exit__(None, None, None)
                return
            # ---------------- deconv ----------------
            d_sb = mp.tile([64, 128 * 128], F16, name="d_sb")
            dv = d_sb[:].rearrange("p (h w) -> p h w", w=128)
            with tc.tile_pool(name="psD", bufs=8, space="PSUM") as psD:
                for kl in range(4):
                    k, l = divmod(kl, 2)
                    for ch in range(8):
                        p0 = ch * 8
                        ps = psD.tile([64, 512], F32, tag="psD")
                        nc.tensor.matmul(ps[:], wdt_sb[:, kl*64:(kl+1)*64],
                                         h2v[:, p0:p0+8, :], start=True, stop=True)
                        dst = dv[:, 2*p0+k:2*p0+k+15:2, l::2]
                        if (kl * 8 + ch) % 2 == 0:
                            nc.vector.tensor_scalar(dst, ps[:], 1.0, db_sb[:],
                                                    ALU.mult, ALU.add)
                        else:
                            nc.scalar.activation(dst, ps[:], AF.Identity,
                                                 bias=db_sb[:], scale=1.0)

            # round trip to put H on partitions
            nc.sync.dma_start(scr2[:], dv)
            mid_pool.__exit__(None, None, None)
            if stage <= 6:
                return
            back_pool = tc.tile_pool(name="back", bufs=1)
            bp = back_pool.__enter__()
            dth = bp.tile([128, 64 * 128], F16, name="dth")
            dth_v = dth[:].rearrange("p (o w) -> p o w", w=128)
            scr2_h = scr2[:].rearrange("o h w -> h o w")
            for oq in range(4):
                eng = nc.sync if oq % 2 == 0 else nc.scalar
                eng.dma_start(dth_v[:, oq*16:(oq+1)*16, :],
                              scr2_h[:, oq*16:(oq+1)*16, :])

            # ---------------- IDWT-H on PE, IDWT-W on DVE ----------------
            # Two overlapping 128-row blocks (rows 0:128 and 126:254): HWDGE only
            # stripes a DMA across the 16 SDMA engines when the SBUF partition
            # count is a multiple of 16; 127-row stores serialize on one engine.
            with tc.tile_pool(name="psE", bufs=8, space="PSUM") as psE, \
                 tc.tile_pool(name="gpool", bufs=2) as gpool, \
                 tc.tile_pool(name="twpool", bufs=1) as twpool, \
                 tc.tile_pool(name="opool", bufs=1) as opool:
                for blk in range(2):
                    r0 = 0 if blk == 0 else 126
                    g_t = gpool.tile([128, 8192], F16, tag="g")
                    g_v = g_t[:].rearrange("p (o w) -> p o w", w=128)
                    for nch in range(16):
                        ps = psE.tile([128, 512], F32, tag="psE")
                        nc.tensor.matmul(ps[:], bhw_sb[:, r0:r0+128],
                                         dth[:, nch*512:(nch+1)*512],
                                         start=True, stop=True)
                        dst = g_t[:, nch*512:(nch+1)*512]
                        if nch % 2 == 0:
                            nc.vector.tensor_copy(dst, ps[:])
                        else:
                            nc.scalar.copy(dst, ps[:])
                    # IDWT-W chunked by o (4 x 16 channels): ACT produces the
                    # scaled copies, DVE interleaves, and each chunk's store
                    # starts while the next chunk computes.
                    o_t = opool.tile([128, 64 * 254], F32, tag="o")
                    o_v = o_t[:].rearrange("p (o w) -> p o w", w=254)
                    tw = twpool.tile([128, 8192], F16, tag="tw")
                    tw_v = tw[:].rearrange("p (o w) -> p o w", w=128)
                    twb = twpool.tile([128, 8192], F16, tag="twb")
                    twb_v = twb[:].rearrange("p (o w) -> p o w", w=128)
                    for oc in range(4):
                        osl = slice(oc * 16, (oc + 1) * 16)
                        nc.scalar.activation(tw_v[:, osl, :], g_v[:, osl, :],
                                             AF.Identity, scale=float(REC[2]))
                        nc.scalar.activation(twb_v[:, osl, :], g_v[:, osl, :],
                                             AF.Identity, scale=float(REC[3]))
                        nc.vector.scalar_tensor_tensor(
                            o_v[:, osl, 0:253:2], g_v[:, osl, 1:128], float(REC[0]),
                            tw_v[:, osl, 0:127], ALU.mult, ALU.add)
                        nc.vector.scalar_tensor_tensor(
                            o_v[:, osl, 1:254:2], g_v[:, osl, 1:128], float(REC[1]),
                            twb_v[:, osl, 0:127], ALU.mult, ALU.add)
                        eng = nc.sync if oc % 2 == 0 else nc.scalar
                        eng.dma_start(out_d[:][r0:r0+128, osl, :], o_v[:, osl, :])
            back_pool.__exit__(None, None, None)

        _body()
    nc.compile()
    return nc


def _bn_coeffs(nc, pp, s1b, s2b, cc_pair, rg, cnt, g_sb, b_sb, use_cc, tag):
    """Reduce per-chunk sums, AllReduce across cores, return (scale, bias) [128,1]."""
    ALU = mybir.AluOpType
    sl = pp.tile([128, 2], F32, name=f"bn{tag}_sl")
    nc.vector.tensor_reduce(sl[:, 0:1], s1b[:], mybir.AxisListType.X, ALU.add)
    nc.vector.tensor_reduce(sl[:, 1:2], s2b[:], mybir.AxisListType.X, ALU.add)
    cc_in, cc_out = cc_pair
    sg = pp.tile([128, 2], F32, name=f"bn{tag}_sg")
    if use_cc:
        nc.sync.dma_start(cc_in[:], sl[:])
        nc.gpsimd.collective_compute(
            "AllReduce", ALU.add, replica_groups=rg,
            ins=[cc_in[:]], outs=[cc_out[:]])
        nc.sync.dma_start(sg[:], cc_out[:])
    else:
        nc.vector.tensor_copy(sg[:], sl[:])

    m = pp.tile([128, 1], F32, name=f"bn{tag}_m")
    vpe = pp.tile([128, 1], F32, name=f"bn{tag}_v")
    t0 = pp.tile([128, 1], F32, name=f"bn{tag}_t0")
    y0 = pp.tile([128, 1], F32, name=f"bn{tag}_y0")
    nc.vector.tensor_scalar(m[:], sg[:, 0:1], 1.0 / cnt, None, ALU.mult)
    nc.vector.tensor_tensor(t0[:], m[:], m[:], ALU.mult)          # m^2
    nc.vector.tensor_scalar(vpe[:], sg[:, 1:2], 1.0 / cnt, float(EPS), ALU.mult,
                            ALU.add)                              # E[x^2]+eps
    nc.vector.tensor_tensor(vpe[:], vpe[:], t0[:], ALU.subtract)  # var+eps
    s0 = pp.tile([128, 1], F32, name=f"bn{tag}_s0")
    nc.scalar.activation(s0[:], vpe[:], mybir.ActivationFunctionType.Sqrt)
    nc.vector.reciprocal(y0[:], s0[:])
    sc = pp.tile([128, 1], F32, name=f"bn{tag}_sc")
    bi = pp.tile([128, 1], F32, name=f"bn{tag}_bi")
    nc.vector.tensor_tensor(sc[:], y0[:], g_sb[:], ALU.mult)
    nc.vector.tensor_tensor(t0[:], m[:], sc[:], ALU.mult)
    nc.vector.tensor_tensor(bi[:], b_sb[:], t0[:], ALU.subtract)
    return sc, bi


# ---------------------------------------------------------------- entry point
_CACHE = {}


def kernel(x, conv1_w, conv1_b, bn1_g, bn1_b, conv2_w, conv2_b, bn2_g, bn2_b,
           deconv_w, deconv_b):
    world = N_CORES
    if "nc" not in _CACHE:
        _CACHE["nc"] = build_nc(world)
    nc = _CACHE["nc"]

    consts = pack_consts(np.asarray(conv1_w), np.asarray(conv2_w),
                         np.asarray(deconv_w), np.asarray(deconv_b),
                         np.asarray(bn1_g), np.asarray(bn1_b),
                         np.asarray(bn2_g), np.asarray(bn2_b))
    x = np.asarray(x)
    in_maps = []
    for n in range(world):
        m = {"x": np.ascontiguousarray(x[n])}
        m.update(consts)
        in_maps.append(m)

    res = run_bass_kernel_spmd(
        nc, in_maps, core_ids=list(range(world)),
        trace=bool(int(os.environ.get("WK_TRACE", "0"))))
    out = np.stack([r["out"].transpose(1, 0, 2) for r in res.results], axis=0)
    _CACHE["last_perf"] = res
    return out

